# revision 32
# baseline (speedup 1.0000x reference)
"""Trainium2 Bass kernel for the DeepBayesianFilterBlockDiag loss.

Strategy (8-core SPMD, observation-axis sharded, TRANSPOSED layout):
  - The 152064-dim observation axis is split into 8 shards of 19008 columns,
    padded to 19072 = 149*128 per core.  The HOST pre-transposes each core's
    target shard to [149, 128, 256] (d-chunk, d-in-chunk, row) so the
    observation axis lands on SBUF/PSUM partitions; log_R likewise arrives
    as [128, 149].  W_dec||b_dec stays [65, 19072].
  - Per core:
      * phase 1: Xe = [mu_f + chol(sigma_f) @ eps, 1] and the KL terms
        (tiny per-(b,t,z) 2x2 algebra); -Xe^T [65,256] is the moving
        operand of the decode GEMM (float32r copy).
      * main loop over 75 PSUM banks (2 d-chunks each): PE injects the
        f32r target chunk-pair into the bank via one identity matmul
        (f32r moving streams at 1 col/cycle), then accumulates
        -Xe @ W' per 128-d chunk with W' slices as f32r stationaries
        (no bf16 conversion pass at all).  ACT squares each chunk
        IN-PLACE in PSUM with accum_out, yielding per-partition
        sums-of-squares directly into acc[:, chunk] — no separate
        subtract, no square tensor in SBUF, no colsum matmuls.
      * epilogue: sse = sum(acc * exp(-2 log_R)), plus sum(log_R) and the
        KL partial, emitted as a [3] vector.
  - Host combines the 8 partial vectors into the final scalar loss.
"""

import math

import numpy as np

import concourse.bass as bass
import concourse.mybir as mybir
import concourse.tile as tile
from concourse.bass_utils import run_bass_kernel_spmd
from concourse.masks import make_identity

F32 = mybir.dt.float32
F32R = mybir.dt.float32r
BF16 = mybir.dt.bfloat16
AF = mybir.ActivationFunctionType
OP = mybir.AluOpType

B, T, Z, DIM = 4, 64, 32, 2
ROWS = B * T          # 256
LAT = Z * DIM         # 64
LATP = LAT + 1        # 65 (ones row folds in b_dec)
D_OBS = 152064
NCORES = 8
DC = D_OBS // NCORES  # 19008 obs columns per core
NCC = 150             # 128-wide d-chunks per core (19200 = padded)
DCP = NCC * 128       # 19200
SEG = 16              # d-chunks per target DMA segment (8 banks)
NBANK = NCC // 2      # 75 psum banks of 2 chunks
# bank residues (mod 5): {0,1} -> ACT square+accum per chunk,
# {2,3,4} -> one DVE bn_stats per bank
ACT_BRES = (0, 1)
BN_BRES = (2, 3, 4)
NBGRP = NBANK // 5    # 15 bank groups of 5
TP_BUFS = 3
DPS_BUFS = 6

MAX_DRAIN_WAITS = 1


def _split_multi_waits(nc, max_waits=1):
    """walrus' per-instruction sync encoding only fits one wait; move extra
    waits emitted by Tile onto NOPs inserted just before the instruction on
    the same engine (same semantics: engine blocks on all of them in order).
    """
    k = 0
    for f in nc.m.functions:
        for blk in f.blocks:
            il = blk.instructions
            i = 0
            while i < len(il):
                inst = il[i]
                si = inst.sync_info
                if si is not None and len(si.on_wait) > max_waits:
                    waits = list(si.on_wait)
                    inst.sync_info = mybir.SyncInfo(
                        on_wait=waits[-max_waits:], on_update=list(si.on_update)
                    )
                    extra = waits[:-max_waits]
                    for j in range(0, len(extra), max_waits):
                        nop = mybir.InstEventSemaphore(
                            name=f"{inst.name}-w{k}",
                            engine=inst.engine,
                            sync_info=mybir.SyncInfo(
                                on_wait=extra[j : j + max_waits], on_update=[]
                            ),
                        )
                        k += 1
                        il.insert(i, nop)
                        i += 1
                i += 1


def _comp4(t, mg, idx):
    # [128, 2, 128] tile -> [128, 32] view of 2x2-block component idx
    return t[:, mg, :].rearrange("p (z k) -> p z k", k=4)[:, :, idx]


def _comp2(t, mg, idx):
    return t[:, mg, :].rearrange("p (z k) -> p z k", k=2)[:, :, idx]


def build_nc(reps: int = 1, split_waits: bool = True, dup: int = 1):
    nc = bass.Bass("TRN2")
    tgt = nc.dram_tensor("tgt", [NCC, 128, ROWS], BF16, kind="ExternalInput")
    wb = nc.dram_tensor("wb", [LATP, DCP], BF16, kind="ExternalInput")
    lrt = nc.dram_tensor("log_r_t", [128, NCC], F32, kind="ExternalInput")
    muf = nc.dram_tensor("mu_f", [ROWS, LAT], F32, kind="ExternalInput")
    sgf = nc.dram_tensor("sig_f", [ROWS, 4 * Z], F32, kind="ExternalInput")
    mup = nc.dram_tensor("mu_p", [ROWS, LAT], F32, kind="ExternalInput")
    sgp = nc.dram_tensor("sig_p", [ROWS, 4 * Z], F32, kind="ExternalInput")
    eps = nc.dram_tensor("eps", [ROWS, LAT], F32, kind="ExternalInput")
    out = nc.dram_tensor("out", [6], F32, kind="ExternalOutput")

    with tile.TileContext(nc) as tc:
        with (
            tc.tile_pool(name="big", bufs=1) as big,
            tc.tile_pool(name="tp", bufs=TP_BUFS) as tpool,
            tc.tile_pool(name="small", bufs=1) as small,
            tc.tile_pool(name="pp2", bufs=2) as pp2,
            tc.tile_pool(name="dps", bufs=DPS_BUFS, space="PSUM") as dpsum,
            tc.tile_pool(name="smallps", bufs=1, space="PSUM") as smallps,
        ):
            # loop-invariant constants, built once
            identf = small.tile([128, 128], F32)
            make_identity(nc, identf)
            ident = small.tile([128, 128], BF16)
            nc.gpsimd.tensor_copy(ident, identf)
            ones = small.tile([128, 1], F32)
            nc.vector.memset(ones, 1.0)
            consts = (identf, ident, ones)
            if reps == 1:
                for _ in range(dup):
                    _body(nc, tc, big, tpool, small, pp2, dpsum, smallps, consts,
                          tgt, wb, lrt, muf, sgf, mup, sgp, eps, out)
            else:
                with tc.For_i(0, reps, 1):
                    for _ in range(dup):
                        _body(nc, tc, big, tpool, small, pp2, dpsum, smallps, consts,
                              tgt, wb, lrt, muf, sgf, mup, sgp, eps, out)
    if split_waits:
        # needed for the walrus/HW path; CoreSim wants the raw form
        _split_multi_waits(nc)
    return nc


def _body(nc, tc, big, tpool, small, pp2, dpsum, smallps, consts,
          tgt, wb, lrt, muf, sgf, mup, sgp, eps, out):
    identf, ident, ones = consts

    # ---- small inputs (SWDGE on the idle Pool queue: issued early so the
    # next rep's phase 1 can overlap this rep's main loop) ----
    sigf_s = small.tile([128, 2, 4 * Z], F32)
    sigp_s = small.tile([128, 2, 4 * Z], F32)
    muf_s = small.tile([128, 2, LAT], F32)
    mup_s = small.tile([128, 2, LAT], F32)
    eps_s = small.tile([128, 2, LAT], F32)
    for mg in range(2):
        ve = nc.gpsimd if mg == 0 else nc.vector
        rs = slice(mg * 128, (mg + 1) * 128)
        nc.sync.dma_start(out=sigf_s[:, mg, :], in_=sgf[rs, :])
        nc.sync.dma_start(out=sigp_s[:, mg, :], in_=sgp[rs, :])
        nc.sync.dma_start(out=muf_s[:, mg, :], in_=muf[rs, :])
        nc.sync.dma_start(out=mup_s[:, mg, :], in_=mup[rs, :])
        nc.sync.dma_start(out=eps_s[:, mg, :], in_=eps[rs, :])
    lrt_s = pp2.tile([128, NCC], F32)
    nc.sync.dma_start(out=lrt_s, in_=lrt[:, :])

    # ---- phase 1: Xe (cholesky sample) + KL, per 128-row group ----
    lhsT = pp2.tile([LATP, 256], F32)
    nc.gpsimd.memset(lhsT[LAT:LATP, :], -1.0)
    kl2 = pp2.tile([128, 2], F32)

    for mg in range(2):
        af = _comp4(sigf_s, mg, 0)
        bf = _comp4(sigf_s, mg, 1)
        cf = _comp4(sigf_s, mg, 2)
        df = _comp4(sigf_s, mg, 3)
        aq = _comp4(sigp_s, mg, 0)
        bq = _comp4(sigp_s, mg, 1)
        cq = _comp4(sigp_s, mg, 2)
        dq = _comp4(sigp_s, mg, 3)

        # cholesky: l11 = sqrt(a); l21 = c/l11; l22 = sqrt(d - l21^2)
        l11 = small.tile([128, Z], F32)
        nc.scalar.sqrt(l11, af)
        r11 = small.tile([128, Z], F32)
        nc.vector.reciprocal(r11, l11)
        l21 = small.tile([128, Z], F32)
        ve.tensor_mul(l21, cf, r11)
        tmp0 = small.tile([128, Z], F32)
        ve.tensor_mul(tmp0, l21, l21)
        ve.tensor_sub(tmp0, df, tmp0)
        l22 = small.tile([128, Z], F32)
        nc.scalar.sqrt(l22, tmp0)

        e1 = _comp2(eps_s, mg, 0)
        e2 = _comp2(eps_s, mg, 1)
        m1 = _comp2(muf_s, mg, 0)
        m2 = _comp2(muf_s, mg, 1)

        xew = small.tile([128, LAT], F32)
        x1v = xew.rearrange("p (z k) -> p z k", k=2)[:, :, 0]
        x2v = xew.rearrange("p (z k) -> p z k", k=2)[:, :, 1]
        tA = small.tile([128, Z], F32)
        ve.tensor_mul(tA, l11, e1)
        ve.tensor_add(x1v, tA, m1)
        tB = small.tile([128, Z], F32)
        ve.tensor_mul(tB, l21, e1)
        tC = small.tile([128, Z], F32)
        ve.tensor_mul(tC, l22, e2)
        ve.tensor_add(tB, tB, tC)
        ve.tensor_add(x2v, tB, m2)

        tps = smallps.tile([LAT, 128], F32, tag="sps")
        nc.tensor.transpose(tps, xew, identf)
        nc.scalar.mul(lhsT[0:LAT, mg * 128 : (mg + 1) * 128], tps, -1.0)

        # KL pieces
        detq = small.tile([128, Z], F32)
        tD = small.tile([128, Z], F32)
        ve.tensor_mul(detq, aq, dq)
        ve.tensor_mul(tD, bq, cq)
        ve.tensor_sub(detq, detq, tD)
        detp = small.tile([128, Z], F32)
        ve.tensor_mul(detp, af, df)
        ve.tensor_mul(tD, bf, cf)
        ve.tensor_sub(detp, detp, tD)
        rdq = small.tile([128, Z], F32)
        nc.vector.reciprocal(rdq, detq)

        # trace numerator: dq*af - bq*bf - cq*cf + aq*df
        tn = small.tile([128, Z], F32)
        ve.tensor_mul(tn, dq, af)
        ve.tensor_mul(tD, aq, df)
        ve.tensor_add(tn, tn, tD)
        ve.tensor_mul(tD, bq, bf)
        ve.tensor_sub(tn, tn, tD)
        ve.tensor_mul(tD, cq, cf)
        ve.tensor_sub(tn, tn, tD)

        # quad numerator: dq*d1^2 - (bq+cq)*d1*d2 + aq*d2^2
        p1 = _comp2(mup_s, mg, 0)
        p2 = _comp2(mup_s, mg, 1)
        d1 = small.tile([128, Z], F32)
        ve.tensor_sub(d1, p1, m1)
        d2 = small.tile([128, Z], F32)
        ve.tensor_sub(d2, p2, m2)
        qn = small.tile([128, Z], F32)
        ve.tensor_mul(tD, d1, d1)
        ve.tensor_mul(qn, dq, tD)
        ve.tensor_mul(tD, d2, d2)
        ve.tensor_mul(tD, aq, tD)
        ve.tensor_add(qn, qn, tD)
        ve.tensor_mul(tD, d1, d2)
        tE = small.tile([128, Z], F32)
        ve.tensor_add(tE, bq, cq)
        ve.tensor_mul(tD, tD, tE)
        ve.tensor_sub(qn, qn, tD)

        klv = small.tile([128, Z], F32)
        ve.tensor_add(klv, tn, qn)
        ve.tensor_mul(klv, klv, rdq)
        # + ln(detq) - ln(detp)
        nc.scalar.activation(tD, detq, AF.Ln)
        ve.tensor_add(klv, klv, tD)
        nc.scalar.activation(tD, detp, AF.Ln)
        ve.tensor_sub(klv, klv, tD)
        nc.vector.reduce_sum(out=kl2[:, mg : mg + 1], in_=klv, axis=mybir.AxisListType.X)

    # moving operand of the decode GEMM: -Xe^T as bf16
    lhsT_r = pp2.tile([LATP, 256], BF16)
    nc.gpsimd.tensor_copy(lhsT_r, lhsT)

    # w = exp(-2 log_R), already [128, NCC] on partitions
    w150 = pp2.tile([128, NCC], F32)
    nc.scalar.activation(w150, lrt_s, AF.Exp, scale=-2.0)

    # ---- W' (with b_dec row) resident in SBUF as f32r, on the scalar ring,
    # interleaved with the target segments on the same (sync) ring ----
    wb_s = big.tile([LATP, DCP], BF16)
    WBSEG = 3840
    wb_offs = list(range(0, DCP, WBSEG))

    # per-chunk sums of squares: ACT banks accumulate into acc columns; DVE
    # banks leave bn_stats moments (per bank, 2 chunks) to recover later
    acc = pp2.tile([128, NCC], F32)
    stats = pp2.tile([128, 3, NBGRP, 2, 6], F32)

    def issue_wb(upto):
        while wb_offs and wb_offs[0] < upto:
            woff = wb_offs.pop(0)
            ww = min(WBSEG, DCP - woff)
            nc.sync.dma_start(
                out=wb_s[:, woff : woff + ww],
                in_=wb[:, woff : woff + ww],
            )

    # ---- phase 2: main loop over target segments / psum banks ----
    issue_wb(2 * WBSEG)  # wb segs 0-1 up front
    for s0 in range(0, NCC, SEG):
        g = min(SEG, NCC - s0)
        t_s = tpool.tile([128, SEG, ROWS], BF16)
        nc.sync.dma_start(
            out=t_s[:, 0:g, :],
            in_=tgt[s0 : s0 + g, :, :].rearrange("g p r -> p g r"),
        )
        # keep the wb stream ~2 segments ahead of the mains consumers
        issue_wb((s0 + 2 * SEG) * 128)
        for b0 in range(0, g, 2):
            dps = dpsum.tile([128, 512], F32)
            nc.tensor.matmul(
                dps,
                lhsT=ident,
                rhs=t_s[:, b0 : b0 + 2, :].rearrange("p g r -> p (g r)"),
                start=True,
                stop=False,
            )
            for c in range(2):
                ch = s0 + b0 + c
                nc.tensor.matmul(
                    dps[:, c * ROWS : (c + 1) * ROWS],
                    lhsT=wb_s[:, ch * 128 : (ch + 1) * 128],
                    rhs=lhsT_r,
                    start=False,
                    stop=(c == 1),
                )
            bank = (s0 + b0) // 2
            rb = bank % 5
            if rb in ACT_BRES:
                for c in range(2):
                    ch = s0 + b0 + c
                    pch = dps[:, c * ROWS : (c + 1) * ROWS]
                    nc.scalar.activation(
                        pch, pch, AF.Square, accum_out=acc[:, ch : ch + 1]
                    )
            else:
                ri = BN_BRES.index(rb)
                for c in range(2):
                    nc.vector.bn_stats(
                        stats[:, ri, bank // 5, c, :],
                        dps[:, c * ROWS : (c + 1) * ROWS],
                    )

    # ---- phase 3: epilogue ----
    # bn moment recovery on Pool, scattered into acc at the bn chunk columns;
    # then one weighted reduce covers ACT and bn chunks alike.
    # combo columns: 0 = sse, 4 = sum(logR), 5 = kl_raw (1-3 spare)
    combo = pp2.tile([128, 6], F32)
    nc.gpsimd.memset(combo[:, 1:4], 0.0)
    accv = acc.rearrange("p (g rb k) -> p g rb k", rb=5, k=2)
    tb1 = pp2.tile([128, NBGRP, 2], F32)
    tb2 = pp2.tile([128, NBGRP, 2], F32)
    for ri, rb in enumerate(BN_BRES):
        me = stats[:, ri, :, :, 1]
        m2e = stats[:, ri, :, :, 2]
        mo = stats[:, ri, :, :, 4]
        m2o = stats[:, ri, :, :, 5]
        nc.gpsimd.tensor_mul(tb1, me, me)
        nc.gpsimd.tensor_mul(tb2, mo, mo)
        nc.gpsimd.tensor_add(tb1, tb1, tb2)
        nc.gpsimd.tensor_add(tb2, m2e, m2o)
        # sq = 128*(me^2+mo^2) + (m2e+m2o)
        nc.vector.scalar_tensor_tensor(
            accv[:, :, rb, :], tb1, 128.0, tb2, op0=OP.mult, op1=OP.add
        )
    prod = pp2.tile([128, NCC], F32)
    nc.gpsimd.tensor_mul(prod, acc, w150)
    nc.vector.reduce_sum(out=combo[:, 0:1], in_=prod, axis=mybir.AxisListType.X)
    nc.vector.reduce_sum(out=combo[:, 4:5], in_=lrt_s, axis=mybir.AxisListType.X)
    nc.gpsimd.tensor_add(combo[:, 5:6], kl2[:, 0:1], kl2[:, 1:2])

    fps = smallps.tile([6, 1], F32, tag="sps")
    nc.tensor.matmul(fps, lhsT=combo, rhs=ones, start=True, stop=True)
    res = pp2.tile([6, 1], F32)
    nc.scalar.copy(res, fps)
    nc.sync.dma_start(out=out[:].rearrange("(p f) -> p f", f=1), in_=res)


_CACHED_NC = {}


def _get_nc(reps: int = 1):
    if reps not in _CACHED_NC:
        _CACHED_NC[reps] = build_nc(reps)
    return _CACHED_NC[reps]


def make_in_maps(mu_filtered, sigma_filtered, mu_pred, sigma_pred, target,
                 W_dec, b_dec, log_R, eps):
    tgt = np.asarray(target, dtype=np.float32).reshape(ROWS, D_OBS)
    wbf = np.concatenate(
        [np.asarray(W_dec, dtype=np.float32),
         np.asarray(b_dec, dtype=np.float32)[None, :]], axis=0
    )
    lr = np.asarray(log_R, dtype=np.float32)
    smalls = {
        "mu_f": np.ascontiguousarray(
            np.asarray(mu_filtered, dtype=np.float32).reshape(ROWS, LAT)),
        "sig_f": np.ascontiguousarray(
            np.asarray(sigma_filtered, dtype=np.float32).reshape(ROWS, 4 * Z)),
        "mu_p": np.ascontiguousarray(
            np.asarray(mu_pred, dtype=np.float32).reshape(ROWS, LAT)),
        "sig_p": np.ascontiguousarray(
            np.asarray(sigma_pred, dtype=np.float32).reshape(ROWS, 4 * Z)),
        "eps": np.ascontiguousarray(
            np.asarray(eps, dtype=np.float32).reshape(ROWS, LAT)),
    }
    import ml_dtypes

    bf16 = ml_dtypes.bfloat16
    in_maps = []
    for c in range(NCORES):
        sl = slice(c * DC, (c + 1) * DC)
        tgt_t = np.zeros((DCP, ROWS), dtype=bf16)
        tgt_t[:DC] = tgt[:, sl].T.astype(bf16)
        wbp = np.zeros((LATP, DCP), dtype=bf16)
        wbp[:, :DC] = wbf[:, sl].astype(bf16)
        lrp = np.zeros(DCP, dtype=np.float32)
        lrp[:DC] = lr[sl]
        in_maps.append({
            **smalls,
            "tgt": np.ascontiguousarray(tgt_t.reshape(NCC, 128, ROWS)),
            "wb": np.ascontiguousarray(wbp),
            "log_r_t": np.ascontiguousarray(lrp.reshape(NCC, 128).T),
        })
    return in_maps


def combine(results):
    sse = 0.0
    slr = 0.0
    for c in range(NCORES):
        v = results[c]["out"]
        sse += float(v[0]) + float(v[1]) + float(v[2]) + float(v[3])
        slr += float(v[4])
    klraw = float(results[0]["out"][5])
    n_tot = ROWS * D_OBS
    loss_integral = 0.5 * (
        n_tot * math.log(2.0 * math.pi) + 2.0 * ROWS * slr + sse
    ) / B
    loss_kl = 0.5 * (klraw - 2.0 * B * T * Z) / B
    return np.float32(loss_integral + loss_kl)


def kernel(mu_filtered, sigma_filtered, mu_pred, sigma_pred, target,
           W_dec, b_dec, log_R, eps):
    nc = _get_nc(1)
    in_maps = make_in_maps(mu_filtered, sigma_filtered, mu_pred, sigma_pred,
                           target, W_dec, b_dec, log_R, eps)
    res = run_bass_kernel_spmd(nc, in_maps, core_ids=list(range(NCORES)))
    return combine(res.results)


# revision 33
# speedup vs baseline: 1.1264x; 1.1264x over previous
"""Trainium2 Bass kernel for the DeepBayesianFilterBlockDiag loss.

Strategy (8-core SPMD, observation-axis sharded, TRANSPOSED layout):
  - The 152064-dim observation axis is split into 8 shards of 19008 columns,
    padded to 19200 = 150*128 per core.  The HOST pre-transposes each core's
    target shard to [150, 128, 256] (d-chunk, d-in-chunk, row) bf16 so the
    observation axis lands on SBUF/PSUM partitions; log_R arrives as
    [128, 150]; W_dec||b_dec as [65, 19200] bf16.  bf16 halves HBM traffic
    (the hard floor) and is far inside the 2e-2 tolerance.
  - Per core:
      * phase 1: Xe = [mu_f + chol(sigma_f) @ eps, 1] and the KL terms
        (tiny per-(b,t,z) 2x2 algebra, split over Pool/DVE so the two
        row-group chains run in parallel); -Xe^T [65,256] bf16 is the
        moving operand of the decode GEMM.
      * main loop over 75 PSUM banks (2 d-chunks each): PE injects the
        bf16 target chunk-pair into the bank with one identity matmul,
        then accumulates -Xe @ W' per 128-d chunk (W' bf16 slices as
        stationary).  Residual d^T = t - rec sits in PSUM with d on
        partitions.  Banks alternate (period 5): 2/5 of banks use ACT
        Square+accum_out per chunk (in-place in PSUM, per-partition sums
        land directly in acc columns); 3/5 use DVE bn_stats per chunk,
        whose moments are recovered to sums-of-squares in the epilogue
        (sum d^2 = M2_even + 128*mean_even^2 + M2_odd + 128*mean_odd^2)
        and scattered into the same acc columns.
      * epilogue: sse = sum(acc * exp(-2 log_R)) via one weighted reduce,
        plus sum(log_R) and the KL partial; a PE ones-matmul reduces the
        [128,6] combo over partitions; out is a [6] vector.
  - Host combines the 8 partial vectors into the final scalar loss.
"""

import math

import numpy as np

import concourse.bass as bass
import concourse.mybir as mybir
import concourse.tile as tile
from concourse.bass_utils import run_bass_kernel_spmd
from concourse.masks import make_identity

F32 = mybir.dt.float32
F32R = mybir.dt.float32r
BF16 = mybir.dt.bfloat16
AF = mybir.ActivationFunctionType
OP = mybir.AluOpType

B, T, Z, DIM = 4, 64, 32, 2
ROWS = B * T          # 256
LAT = Z * DIM         # 64
LATP = LAT + 1        # 65 (ones row folds in b_dec)
D_OBS = 152064
NCORES = 8
DC = D_OBS // NCORES  # 19008 obs columns per core
NCC = 150             # 128-wide d-chunks per core (19200 = padded)
DCP = NCC * 128       # 19200
SEG = 16              # d-chunks per target DMA segment (8 banks)
NBANK = NCC // 2      # 75 psum banks of 2 chunks
# bank residues (mod 5): {0,1} -> ACT square+accum per chunk,
# {2,3,4} -> one DVE bn_stats per bank
ACT_BRES = (0, 1)
BN_BRES = (2, 3, 4)
NBGRP = NBANK // 5    # 15 bank groups of 5
TP_BUFS = 3
DPS_BUFS = 6

MAX_DRAIN_WAITS = 1


def _split_multi_waits(nc, max_waits=1):
    """walrus' per-instruction sync encoding only fits one wait; move extra
    waits emitted by Tile onto NOPs inserted just before the instruction on
    the same engine (same semantics: engine blocks on all of them in order).
    """
    k = 0
    for f in nc.m.functions:
        for blk in f.blocks:
            il = blk.instructions
            i = 0
            while i < len(il):
                inst = il[i]
                si = inst.sync_info
                if si is not None and len(si.on_wait) > max_waits:
                    waits = list(si.on_wait)
                    inst.sync_info = mybir.SyncInfo(
                        on_wait=waits[-max_waits:], on_update=list(si.on_update)
                    )
                    extra = waits[:-max_waits]
                    for j in range(0, len(extra), max_waits):
                        nop = mybir.InstEventSemaphore(
                            name=f"{inst.name}-w{k}",
                            engine=inst.engine,
                            sync_info=mybir.SyncInfo(
                                on_wait=extra[j : j + max_waits], on_update=[]
                            ),
                        )
                        k += 1
                        il.insert(i, nop)
                        i += 1
                i += 1


def _comp4(t, mg, idx):
    # [128, 2, 128] tile -> [128, 32] view of 2x2-block component idx
    return t[:, mg, :].rearrange("p (z k) -> p z k", k=4)[:, :, idx]


def _comp2(t, mg, idx):
    return t[:, mg, :].rearrange("p (z k) -> p z k", k=2)[:, :, idx]


def build_nc(reps: int = 1, split_waits: bool = True, dup: int = 1):
    nc = bass.Bass("TRN2")
    tgt = nc.dram_tensor("tgt", [NCC, 128, ROWS], BF16, kind="ExternalInput")
    wb = nc.dram_tensor("wb", [LATP, DCP], BF16, kind="ExternalInput")
    lrt = nc.dram_tensor("log_r_t", [128, NCC], F32, kind="ExternalInput")
    muf = nc.dram_tensor("mu_f", [ROWS, LAT], F32, kind="ExternalInput")
    sgf = nc.dram_tensor("sig_f", [ROWS, 4 * Z], F32, kind="ExternalInput")
    mup = nc.dram_tensor("mu_p", [ROWS, LAT], F32, kind="ExternalInput")
    sgp = nc.dram_tensor("sig_p", [ROWS, 4 * Z], F32, kind="ExternalInput")
    eps = nc.dram_tensor("eps", [ROWS, LAT], F32, kind="ExternalInput")
    out = nc.dram_tensor("out", [6], F32, kind="ExternalOutput")

    with tile.TileContext(nc) as tc:
        with (
            tc.tile_pool(name="big", bufs=1) as big,
            tc.tile_pool(name="tp", bufs=TP_BUFS) as tpool,
            tc.tile_pool(name="small", bufs=1) as small,
            tc.tile_pool(name="pp2", bufs=2) as pp2,
            tc.tile_pool(name="dps", bufs=DPS_BUFS, space="PSUM") as dpsum,
            tc.tile_pool(name="smallps", bufs=1, space="PSUM") as smallps,
        ):
            # loop-invariant constants, built once
            identf = small.tile([128, 128], F32)
            make_identity(nc, identf)
            ident = small.tile([128, 128], BF16)
            nc.gpsimd.tensor_copy(ident, identf)
            ones = small.tile([128, 1], F32)
            nc.vector.memset(ones, 1.0)
            consts = (identf, ident, ones)
            if reps == 1:
                for _ in range(dup):
                    _body(nc, tc, big, tpool, small, pp2, dpsum, smallps, consts,
                          tgt, wb, lrt, muf, sgf, mup, sgp, eps, out)
            else:
                with tc.For_i(0, reps, 1):
                    for _ in range(dup):
                        _body(nc, tc, big, tpool, small, pp2, dpsum, smallps, consts,
                              tgt, wb, lrt, muf, sgf, mup, sgp, eps, out)
    if split_waits:
        # needed for the walrus/HW path; CoreSim wants the raw form
        _split_multi_waits(nc)
    return nc


def _body(nc, tc, big, tpool, small, pp2, dpsum, smallps, consts,
          tgt, wb, lrt, muf, sgf, mup, sgp, eps, out):
    identf, ident, ones = consts

    # ---- small inputs (SWDGE on the idle Pool queue: issued early so the
    # next rep's phase 1 can overlap this rep's main loop) ----
    sigf_s = small.tile([128, 2, 4 * Z], F32)
    sigp_s = small.tile([128, 2, 4 * Z], F32)
    muf_s = small.tile([128, 2, LAT], F32)
    mup_s = small.tile([128, 2, LAT], F32)
    eps_s = small.tile([128, 2, LAT], F32)
    for mg in range(2):
        ve = nc.gpsimd if mg == 0 else nc.vector
        rs = slice(mg * 128, (mg + 1) * 128)
        nc.sync.dma_start(out=sigf_s[:, mg, :], in_=sgf[rs, :])
        nc.sync.dma_start(out=sigp_s[:, mg, :], in_=sgp[rs, :])
        nc.sync.dma_start(out=muf_s[:, mg, :], in_=muf[rs, :])
        nc.sync.dma_start(out=mup_s[:, mg, :], in_=mup[rs, :])
        nc.sync.dma_start(out=eps_s[:, mg, :], in_=eps[rs, :])
    lrt_s = pp2.tile([128, NCC], F32)
    nc.sync.dma_start(out=lrt_s, in_=lrt[:, :])

    # ---- phase 1: Xe (cholesky sample) + KL, per 128-row group ----
    lhsT = pp2.tile([LATP, 256], F32)
    nc.gpsimd.memset(lhsT[LAT:LATP, :], -1.0)
    kl2 = pp2.tile([128, 2], F32)

    for mg in range(2):
        af = _comp4(sigf_s, mg, 0)
        bf = _comp4(sigf_s, mg, 1)
        cf = _comp4(sigf_s, mg, 2)
        df = _comp4(sigf_s, mg, 3)
        aq = _comp4(sigp_s, mg, 0)
        bq = _comp4(sigp_s, mg, 1)
        cq = _comp4(sigp_s, mg, 2)
        dq = _comp4(sigp_s, mg, 3)

        # cholesky: l11 = sqrt(a); l21 = c/l11; l22 = sqrt(d - l21^2)
        l11 = small.tile([128, Z], F32)
        nc.scalar.sqrt(l11, af)
        r11 = small.tile([128, Z], F32)
        nc.vector.reciprocal(r11, l11)
        l21 = small.tile([128, Z], F32)
        ve.tensor_mul(l21, cf, r11)
        tmp0 = small.tile([128, Z], F32)
        ve.tensor_mul(tmp0, l21, l21)
        ve.tensor_sub(tmp0, df, tmp0)
        l22 = small.tile([128, Z], F32)
        nc.scalar.sqrt(l22, tmp0)

        e1 = _comp2(eps_s, mg, 0)
        e2 = _comp2(eps_s, mg, 1)
        m1 = _comp2(muf_s, mg, 0)
        m2 = _comp2(muf_s, mg, 1)

        xew = small.tile([128, LAT], F32)
        x1v = xew.rearrange("p (z k) -> p z k", k=2)[:, :, 0]
        x2v = xew.rearrange("p (z k) -> p z k", k=2)[:, :, 1]
        tA = small.tile([128, Z], F32)
        ve.tensor_mul(tA, l11, e1)
        ve.tensor_add(x1v, tA, m1)
        tB = small.tile([128, Z], F32)
        ve.tensor_mul(tB, l21, e1)
        tC = small.tile([128, Z], F32)
        ve.tensor_mul(tC, l22, e2)
        ve.tensor_add(tB, tB, tC)
        ve.tensor_add(x2v, tB, m2)

        tps = smallps.tile([LAT, 128], F32, tag="sps")
        nc.tensor.transpose(tps, xew, identf)
        nc.scalar.mul(lhsT[0:LAT, mg * 128 : (mg + 1) * 128], tps, -1.0)

        # KL pieces
        detq = small.tile([128, Z], F32)
        tD = small.tile([128, Z], F32)
        ve.tensor_mul(detq, aq, dq)
        ve.tensor_mul(tD, bq, cq)
        ve.tensor_sub(detq, detq, tD)
        detp = small.tile([128, Z], F32)
        ve.tensor_mul(detp, af, df)
        ve.tensor_mul(tD, bf, cf)
        ve.tensor_sub(detp, detp, tD)
        rdq = small.tile([128, Z], F32)
        nc.vector.reciprocal(rdq, detq)

        # trace numerator: dq*af - bq*bf - cq*cf + aq*df
        tn = small.tile([128, Z], F32)
        ve.tensor_mul(tn, dq, af)
        ve.tensor_mul(tD, aq, df)
        ve.tensor_add(tn, tn, tD)
        ve.tensor_mul(tD, bq, bf)
        ve.tensor_sub(tn, tn, tD)
        ve.tensor_mul(tD, cq, cf)
        ve.tensor_sub(tn, tn, tD)

        # quad numerator: dq*d1^2 - (bq+cq)*d1*d2 + aq*d2^2
        p1 = _comp2(mup_s, mg, 0)
        p2 = _comp2(mup_s, mg, 1)
        d1 = small.tile([128, Z], F32)
        ve.tensor_sub(d1, p1, m1)
        d2 = small.tile([128, Z], F32)
        ve.tensor_sub(d2, p2, m2)
        qn = small.tile([128, Z], F32)
        ve.tensor_mul(tD, d1, d1)
        ve.tensor_mul(qn, dq, tD)
        ve.tensor_mul(tD, d2, d2)
        ve.tensor_mul(tD, aq, tD)
        ve.tensor_add(qn, qn, tD)
        ve.tensor_mul(tD, d1, d2)
        tE = small.tile([128, Z], F32)
        ve.tensor_add(tE, bq, cq)
        ve.tensor_mul(tD, tD, tE)
        ve.tensor_sub(qn, qn, tD)

        klv = small.tile([128, Z], F32)
        ve.tensor_add(klv, tn, qn)
        ve.tensor_mul(klv, klv, rdq)
        # + ln(detq) - ln(detp)
        nc.scalar.activation(tD, detq, AF.Ln)
        ve.tensor_add(klv, klv, tD)
        nc.scalar.activation(tD, detp, AF.Ln)
        ve.tensor_sub(klv, klv, tD)
        nc.vector.reduce_sum(out=kl2[:, mg : mg + 1], in_=klv, axis=mybir.AxisListType.X)

    # moving operand of the decode GEMM: -Xe^T as bf16
    lhsT_r = pp2.tile([LATP, 256], BF16)
    nc.gpsimd.tensor_copy(lhsT_r, lhsT)

    # w = exp(-2 log_R), already [128, NCC] on partitions
    w150 = pp2.tile([128, NCC], F32)
    nc.scalar.activation(w150, lrt_s, AF.Exp, scale=-2.0)

    # ---- W' (with b_dec row) resident in SBUF as f32r, on the scalar ring,
    # interleaved with the target segments on the same (sync) ring ----
    wb_s = big.tile([LATP, DCP], BF16)
    WBSEG = 3840
    wb_offs = list(range(0, DCP, WBSEG))

    # per-chunk sums of squares: ACT banks accumulate into acc columns; DVE
    # banks leave bn_stats moments (per bank, 2 chunks) to recover later
    acc = pp2.tile([128, NCC], F32)
    stats = pp2.tile([128, 3, NBGRP, 2, 6], F32)

    def issue_wb(upto):
        while wb_offs and wb_offs[0] < upto:
            woff = wb_offs.pop(0)
            ww = min(WBSEG, DCP - woff)
            nc.sync.dma_start(
                out=wb_s[:, woff : woff + ww],
                in_=wb[:, woff : woff + ww],
            )

    # ---- phase 2: main loop over target segments / psum banks ----
    issue_wb(2 * WBSEG)  # wb segs 0-1 up front
    for s0 in range(0, NCC, SEG):
        g = min(SEG, NCC - s0)
        t_s = tpool.tile([128, SEG, ROWS], BF16)
        nc.sync.dma_start(
            out=t_s[:, 0:g, :],
            in_=tgt[s0 : s0 + g, :, :].rearrange("g p r -> p g r"),
        )
        # keep the wb stream ~2 segments ahead of the mains consumers
        issue_wb((s0 + 2 * SEG) * 128)
        for b0 in range(0, g, 2):
            dps = dpsum.tile([128, 512], F32)
            nc.tensor.matmul(
                dps,
                lhsT=ident,
                rhs=t_s[:, b0 : b0 + 2, :].rearrange("p g r -> p (g r)"),
                start=True,
                stop=False,
            )
            for c in range(2):
                ch = s0 + b0 + c
                nc.tensor.matmul(
                    dps[:, c * ROWS : (c + 1) * ROWS],
                    lhsT=wb_s[:, ch * 128 : (ch + 1) * 128],
                    rhs=lhsT_r,
                    start=False,
                    stop=(c == 1),
                )
            bank = (s0 + b0) // 2
            rb = bank % 5
            if rb in ACT_BRES:
                for c in range(2):
                    ch = s0 + b0 + c
                    pch = dps[:, c * ROWS : (c + 1) * ROWS]
                    nc.scalar.activation(
                        pch, pch, AF.Square, accum_out=acc[:, ch : ch + 1]
                    )
            else:
                ri = BN_BRES.index(rb)
                for c in range(2):
                    nc.vector.bn_stats(
                        stats[:, ri, bank // 5, c, :],
                        dps[:, c * ROWS : (c + 1) * ROWS],
                    )

    # ---- phase 3: epilogue ----
    # bn moment recovery on Pool, scattered into acc at the bn chunk columns;
    # then one weighted reduce covers ACT and bn chunks alike.
    # combo columns: 0 = sse, 4 = sum(logR), 5 = kl_raw (1-3 spare)
    combo = pp2.tile([128, 6], F32)
    nc.gpsimd.memset(combo[:, 1:4], 0.0)
    accv = acc.rearrange("p (g rb k) -> p g rb k", rb=5, k=2)
    tb1 = pp2.tile([128, NBGRP, 2], F32)
    tb2 = pp2.tile([128, NBGRP, 2], F32)
    for ri, rb in enumerate(BN_BRES):
        me = stats[:, ri, :, :, 1]
        m2e = stats[:, ri, :, :, 2]
        mo = stats[:, ri, :, :, 4]
        m2o = stats[:, ri, :, :, 5]
        nc.gpsimd.tensor_mul(tb1, me, me)
        nc.gpsimd.tensor_mul(tb2, mo, mo)
        nc.gpsimd.tensor_add(tb1, tb1, tb2)
        nc.gpsimd.tensor_add(tb2, m2e, m2o)
        # sq = 128*(me^2+mo^2) + (m2e+m2o)
        nc.vector.scalar_tensor_tensor(
            accv[:, :, rb, :], tb1, 128.0, tb2, op0=OP.mult, op1=OP.add
        )
    prod = pp2.tile([128, NCC], F32)
    nc.gpsimd.tensor_mul(prod, acc, w150)
    nc.vector.reduce_sum(out=combo[:, 0:1], in_=prod, axis=mybir.AxisListType.X)
    nc.vector.reduce_sum(out=combo[:, 4:5], in_=lrt_s, axis=mybir.AxisListType.X)
    nc.gpsimd.tensor_add(combo[:, 5:6], kl2[:, 0:1], kl2[:, 1:2])

    fps = smallps.tile([6, 1], F32, tag="sps")
    nc.tensor.matmul(fps, lhsT=combo, rhs=ones, start=True, stop=True)
    res = pp2.tile([6, 1], F32)
    nc.scalar.copy(res, fps)
    nc.sync.dma_start(out=out[:].rearrange("(p f) -> p f", f=1), in_=res)


_CACHED_NC = {}


def _get_nc(reps: int = 1):
    if reps not in _CACHED_NC:
        _CACHED_NC[reps] = build_nc(reps)
    return _CACHED_NC[reps]


def make_in_maps(mu_filtered, sigma_filtered, mu_pred, sigma_pred, target,
                 W_dec, b_dec, log_R, eps):
    tgt = np.asarray(target, dtype=np.float32).reshape(ROWS, D_OBS)
    wbf = np.concatenate(
        [np.asarray(W_dec, dtype=np.float32),
         np.asarray(b_dec, dtype=np.float32)[None, :]], axis=0
    )
    lr = np.asarray(log_R, dtype=np.float32)
    smalls = {
        "mu_f": np.ascontiguousarray(
            np.asarray(mu_filtered, dtype=np.float32).reshape(ROWS, LAT)),
        "sig_f": np.ascontiguousarray(
            np.asarray(sigma_filtered, dtype=np.float32).reshape(ROWS, 4 * Z)),
        "mu_p": np.ascontiguousarray(
            np.asarray(mu_pred, dtype=np.float32).reshape(ROWS, LAT)),
        "sig_p": np.ascontiguousarray(
            np.asarray(sigma_pred, dtype=np.float32).reshape(ROWS, 4 * Z)),
        "eps": np.ascontiguousarray(
            np.asarray(eps, dtype=np.float32).reshape(ROWS, LAT)),
    }
    import ml_dtypes

    bf16 = ml_dtypes.bfloat16
    in_maps = []
    for c in range(NCORES):
        sl = slice(c * DC, (c + 1) * DC)
        tgt_t = np.zeros((DCP, ROWS), dtype=bf16)
        tgt_t[:DC] = tgt[:, sl].T.astype(bf16)
        wbp = np.zeros((LATP, DCP), dtype=bf16)
        wbp[:, :DC] = wbf[:, sl].astype(bf16)
        lrp = np.zeros(DCP, dtype=np.float32)
        lrp[:DC] = lr[sl]
        in_maps.append({
            **smalls,
            "tgt": np.ascontiguousarray(tgt_t.reshape(NCC, 128, ROWS)),
            "wb": np.ascontiguousarray(wbp),
            "log_r_t": np.ascontiguousarray(lrp.reshape(NCC, 128).T),
        })
    return in_maps


def combine(results):
    sse = 0.0
    slr = 0.0
    for c in range(NCORES):
        v = results[c]["out"]
        sse += float(v[0]) + float(v[1]) + float(v[2]) + float(v[3])
        slr += float(v[4])
    klraw = float(results[0]["out"][5])
    n_tot = ROWS * D_OBS
    loss_integral = 0.5 * (
        n_tot * math.log(2.0 * math.pi) + 2.0 * ROWS * slr + sse
    ) / B
    loss_kl = 0.5 * (klraw - 2.0 * B * T * Z) / B
    return np.float32(loss_integral + loss_kl)


def kernel(mu_filtered, sigma_filtered, mu_pred, sigma_pred, target,
           W_dec, b_dec, log_R, eps):
    nc = _get_nc(1)
    in_maps = make_in_maps(mu_filtered, sigma_filtered, mu_pred, sigma_pred,
                           target, W_dec, b_dec, log_R, eps)
    res = run_bass_kernel_spmd(nc, in_maps, core_ids=list(range(NCORES)))
    return combine(res.results)


# revision 40
# speedup vs baseline: 1.3436x; 1.1928x over previous
"""Trainium2 Bass kernel for the DeepBayesianFilterBlockDiag loss.

Strategy (8-core SPMD, observation-axis sharded, TRANSPOSED layout):
  - The 152064-dim observation axis is split into 8 shards of 19008 columns,
    padded to 19200 = 150*128 per core.  The HOST pre-transposes each core's
    target shard to [75, 128, 512] (bank, d-in-chunk, chunk-pair x row)
    fp8-e4m3 so the observation axis lands on SBUF/PSUM partitions with
    512B-contiguous DMA lines; log_R arrives as [128, 150]; W_dec||b_dec as
    [65, 19200] fp8.  1-byte operands cut HBM traffic (the hard floor) 4x
    vs f32; the resulting ~5e-4 loss error is far inside the 2e-2
    tolerance.
  - Per core:
      * phase 1: Xe = [mu_f + chol(sigma_f) @ eps, 1] and the KL terms
        (tiny per-(b,t,z) 2x2 algebra, split over Pool/DVE so the two
        row-group chains run in parallel); -Xe^T [65,256] fp8 is the
        moving operand of the decode GEMM.
      * main loop over 75 PSUM banks (2 d-chunks each): PE injects the
        fp8 target chunk-pair into the bank with one identity matmul,
        then accumulates -Xe @ W' per 128-d chunk (W' fp8 slices as
        stationary, -Xe^T fp8 moving).  Residual d^T = t - rec sits in PSUM with d on
        partitions.  Banks alternate (period 5): 2/5 of banks use ACT
        Square+accum_out per chunk (in-place in PSUM, per-partition sums
        land directly in acc columns); 3/5 use DVE bn_stats per chunk,
        whose moments are recovered to sums-of-squares in the epilogue
        (sum d^2 = M2_even + 128*mean_even^2 + M2_odd + 128*mean_odd^2)
        and scattered into the same acc columns.
      * epilogue: sse = sum(acc * exp(-2 log_R)) via one weighted reduce,
        plus sum(log_R) and the KL partial; a PE ones-matmul reduces the
        [128,6] combo over partitions; out is a [6] vector.
  - Host combines the 8 partial vectors into the final scalar loss.
"""

import math

import numpy as np

import concourse.bass as bass
import concourse.mybir as mybir
import concourse.tile as tile
from concourse.bass_utils import run_bass_kernel_spmd
from concourse.masks import make_identity

F32 = mybir.dt.float32
F32R = mybir.dt.float32r
BF16 = mybir.dt.bfloat16
FP8 = mybir.dt.float8e4
AF = mybir.ActivationFunctionType
OP = mybir.AluOpType

B, T, Z, DIM = 4, 64, 32, 2
ROWS = B * T          # 256
LAT = Z * DIM         # 64
LATP = LAT + 1        # 65 (ones row folds in b_dec)
D_OBS = 152064
NCORES = 8
DC = D_OBS // NCORES  # 19008 obs columns per core
NCC = 150             # 128-wide d-chunks per core (19200 = padded)
DCP = NCC * 128       # 19200
SEG = 16              # d-chunks per target DMA segment (8 banks)
NBANK = NCC // 2      # 75 psum banks of 2 chunks
# bank residues (mod 5): {0,1} -> ACT square+accum per chunk,
# {2,3,4} -> one DVE bn_stats per bank
ACT_BRES = (0, 1)
BN_BRES = (2, 3, 4)
NBGRP = NBANK // 5    # 15 bank groups of 5
TP_BUFS = 3
DPS_BUFS = 6

MAX_DRAIN_WAITS = 1


def _split_multi_waits(nc, max_waits=1):
    """walrus' per-instruction sync encoding only fits one wait; move extra
    waits emitted by Tile onto NOPs inserted just before the instruction on
    the same engine (same semantics: engine blocks on all of them in order).
    """
    k = 0
    for f in nc.m.functions:
        for blk in f.blocks:
            il = blk.instructions
            i = 0
            while i < len(il):
                inst = il[i]
                si = inst.sync_info
                if si is not None and len(si.on_wait) > max_waits:
                    waits = list(si.on_wait)
                    inst.sync_info = mybir.SyncInfo(
                        on_wait=waits[-max_waits:], on_update=list(si.on_update)
                    )
                    extra = waits[:-max_waits]
                    for j in range(0, len(extra), max_waits):
                        nop = mybir.InstEventSemaphore(
                            name=f"{inst.name}-w{k}",
                            engine=inst.engine,
                            sync_info=mybir.SyncInfo(
                                on_wait=extra[j : j + max_waits], on_update=[]
                            ),
                        )
                        k += 1
                        il.insert(i, nop)
                        i += 1
                i += 1


def _comp4(t, mg, idx):
    # [128, 2, 128] tile -> [128, 32] view of 2x2-block component idx
    return t[:, mg, :].rearrange("p (z k) -> p z k", k=4)[:, :, idx]


def _comp2(t, mg, idx):
    return t[:, mg, :].rearrange("p (z k) -> p z k", k=2)[:, :, idx]


def build_nc(reps: int = 1, split_waits: bool = True, dup: int = 1):
    nc = bass.Bass("TRN2")
    tgt = nc.dram_tensor("tgt", [NBANK, 128, 2 * ROWS], FP8, kind="ExternalInput")
    wb = nc.dram_tensor("wb", [LATP, DCP], FP8, kind="ExternalInput")
    lrt = nc.dram_tensor("log_r_t", [128, NCC], F32, kind="ExternalInput")
    muf = nc.dram_tensor("mu_f", [ROWS, LAT], F32, kind="ExternalInput")
    sgf = nc.dram_tensor("sig_f", [ROWS, 4 * Z], F32, kind="ExternalInput")
    mup = nc.dram_tensor("mu_p", [ROWS, LAT], F32, kind="ExternalInput")
    sgp = nc.dram_tensor("sig_p", [ROWS, 4 * Z], F32, kind="ExternalInput")
    eps = nc.dram_tensor("eps", [ROWS, LAT], F32, kind="ExternalInput")
    out = nc.dram_tensor("out", [6], F32, kind="ExternalOutput")

    with tile.TileContext(nc) as tc:
        with (
            tc.tile_pool(name="big", bufs=1) as big,
            tc.tile_pool(name="tp", bufs=TP_BUFS) as tpool,
            tc.tile_pool(name="small", bufs=1) as small,
            tc.tile_pool(name="pp2", bufs=2) as pp2,
            tc.tile_pool(name="dps", bufs=DPS_BUFS, space="PSUM") as dpsum,
            tc.tile_pool(name="smallps", bufs=1, space="PSUM") as smallps,
        ):
            # loop-invariant constants, built once
            identf = small.tile([128, 128], F32)
            make_identity(nc, identf)
            ident = small.tile([128, 128], FP8)
            nc.gpsimd.tensor_copy(ident, identf)
            ones = small.tile([128, 1], F32)
            nc.vector.memset(ones, 1.0)
            consts = (identf, ident, ones)
            if reps == 1:
                for _ in range(dup):
                    _body(nc, tc, big, tpool, small, pp2, dpsum, smallps, consts,
                          tgt, wb, lrt, muf, sgf, mup, sgp, eps, out)
            else:
                with tc.For_i(0, reps, 1):
                    for _ in range(dup):
                        _body(nc, tc, big, tpool, small, pp2, dpsum, smallps, consts,
                              tgt, wb, lrt, muf, sgf, mup, sgp, eps, out)
    if split_waits:
        # needed for the walrus/HW path; CoreSim wants the raw form
        _split_multi_waits(nc)
    return nc


def _body(nc, tc, big, tpool, small, pp2, dpsum, smallps, consts,
          tgt, wb, lrt, muf, sgf, mup, sgp, eps, out):
    identf, ident, ones = consts

    # ---- small inputs (SWDGE on the idle Pool queue: issued early so the
    # next rep's phase 1 can overlap this rep's main loop) ----
    sigf_s = small.tile([128, 2, 4 * Z], F32)
    sigp_s = small.tile([128, 2, 4 * Z], F32)
    muf_s = small.tile([128, 2, LAT], F32)
    mup_s = small.tile([128, 2, LAT], F32)
    eps_s = small.tile([128, 2, LAT], F32)
    for mg in range(2):
        ve = nc.gpsimd if mg == 0 else nc.vector
        rs = slice(mg * 128, (mg + 1) * 128)
        nc.sync.dma_start(out=sigf_s[:, mg, :], in_=sgf[rs, :])
        nc.sync.dma_start(out=sigp_s[:, mg, :], in_=sgp[rs, :])
        nc.sync.dma_start(out=muf_s[:, mg, :], in_=muf[rs, :])
        nc.sync.dma_start(out=mup_s[:, mg, :], in_=mup[rs, :])
        nc.sync.dma_start(out=eps_s[:, mg, :], in_=eps[rs, :])
    lrt_s = pp2.tile([128, NCC], F32)
    nc.sync.dma_start(out=lrt_s, in_=lrt[:, :])

    # ---- phase 1: Xe (cholesky sample) + KL, per 128-row group ----
    lhsT = pp2.tile([LATP, 256], F32)
    nc.gpsimd.memset(lhsT[LAT:LATP, :], -1.0)
    kl2 = pp2.tile([128, 2], F32)

    for mg in range(2):
        af = _comp4(sigf_s, mg, 0)
        bf = _comp4(sigf_s, mg, 1)
        cf = _comp4(sigf_s, mg, 2)
        df = _comp4(sigf_s, mg, 3)
        aq = _comp4(sigp_s, mg, 0)
        bq = _comp4(sigp_s, mg, 1)
        cq = _comp4(sigp_s, mg, 2)
        dq = _comp4(sigp_s, mg, 3)

        # cholesky: l11 = sqrt(a); l21 = c/l11; l22 = sqrt(d - l21^2)
        l11 = small.tile([128, Z], F32)
        nc.scalar.sqrt(l11, af)
        r11 = small.tile([128, Z], F32)
        nc.vector.reciprocal(r11, l11)
        l21 = small.tile([128, Z], F32)
        ve.tensor_mul(l21, cf, r11)
        tmp0 = small.tile([128, Z], F32)
        ve.tensor_mul(tmp0, l21, l21)
        ve.tensor_sub(tmp0, df, tmp0)
        l22 = small.tile([128, Z], F32)
        nc.scalar.sqrt(l22, tmp0)

        e1 = _comp2(eps_s, mg, 0)
        e2 = _comp2(eps_s, mg, 1)
        m1 = _comp2(muf_s, mg, 0)
        m2 = _comp2(muf_s, mg, 1)

        xew = small.tile([128, LAT], F32)
        x1v = xew.rearrange("p (z k) -> p z k", k=2)[:, :, 0]
        x2v = xew.rearrange("p (z k) -> p z k", k=2)[:, :, 1]
        tA = small.tile([128, Z], F32)
        ve.tensor_mul(tA, l11, e1)
        ve.tensor_add(x1v, tA, m1)
        tB = small.tile([128, Z], F32)
        ve.tensor_mul(tB, l21, e1)
        tC = small.tile([128, Z], F32)
        ve.tensor_mul(tC, l22, e2)
        ve.tensor_add(tB, tB, tC)
        ve.tensor_add(x2v, tB, m2)

        tps = smallps.tile([LAT, 128], F32, tag="sps")
        nc.tensor.transpose(tps, xew, identf)
        nc.scalar.mul(lhsT[0:LAT, mg * 128 : (mg + 1) * 128], tps, -1.0)

        # KL pieces
        detq = small.tile([128, Z], F32)
        tD = small.tile([128, Z], F32)
        ve.tensor_mul(detq, aq, dq)
        ve.tensor_mul(tD, bq, cq)
        ve.tensor_sub(detq, detq, tD)
        detp = small.tile([128, Z], F32)
        ve.tensor_mul(detp, af, df)
        ve.tensor_mul(tD, bf, cf)
        ve.tensor_sub(detp, detp, tD)
        rdq = small.tile([128, Z], F32)
        nc.vector.reciprocal(rdq, detq)

        # trace numerator: dq*af - bq*bf - cq*cf + aq*df
        tn = small.tile([128, Z], F32)
        ve.tensor_mul(tn, dq, af)
        ve.tensor_mul(tD, aq, df)
        ve.tensor_add(tn, tn, tD)
        ve.tensor_mul(tD, bq, bf)
        ve.tensor_sub(tn, tn, tD)
        ve.tensor_mul(tD, cq, cf)
        ve.tensor_sub(tn, tn, tD)

        # quad numerator: dq*d1^2 - (bq+cq)*d1*d2 + aq*d2^2
        p1 = _comp2(mup_s, mg, 0)
        p2 = _comp2(mup_s, mg, 1)
        d1 = small.tile([128, Z], F32)
        ve.tensor_sub(d1, p1, m1)
        d2 = small.tile([128, Z], F32)
        ve.tensor_sub(d2, p2, m2)
        qn = small.tile([128, Z], F32)
        ve.tensor_mul(tD, d1, d1)
        ve.tensor_mul(qn, dq, tD)
        ve.tensor_mul(tD, d2, d2)
        ve.tensor_mul(tD, aq, tD)
        ve.tensor_add(qn, qn, tD)
        ve.tensor_mul(tD, d1, d2)
        tE = small.tile([128, Z], F32)
        ve.tensor_add(tE, bq, cq)
        ve.tensor_mul(tD, tD, tE)
        ve.tensor_sub(qn, qn, tD)

        klv = small.tile([128, Z], F32)
        ve.tensor_add(klv, tn, qn)
        ve.tensor_mul(klv, klv, rdq)
        # + ln(detq) - ln(detp)
        nc.scalar.activation(tD, detq, AF.Ln)
        ve.tensor_add(klv, klv, tD)
        nc.scalar.activation(tD, detp, AF.Ln)
        ve.tensor_sub(klv, klv, tD)
        nc.vector.reduce_sum(out=kl2[:, mg : mg + 1], in_=klv, axis=mybir.AxisListType.X)

    # moving operand of the decode GEMM: -Xe^T as bf16
    lhsT_r = pp2.tile([LATP, 256], FP8)
    nc.gpsimd.tensor_copy(lhsT_r, lhsT)

    # w = exp(-2 log_R), already [128, NCC] on partitions
    w150 = pp2.tile([128, NCC], F32)
    nc.scalar.activation(w150, lrt_s, AF.Exp, scale=-2.0)

    # ---- W' (with b_dec row) resident in SBUF as f32r, on the scalar ring,
    # interleaved with the target segments on the same (sync) ring ----
    wb_s = big.tile([LATP, DCP], FP8)
    WBSEG = 3840
    wb_offs = list(range(0, DCP, WBSEG))

    # per-chunk sums of squares: ACT banks accumulate into acc columns; DVE
    # banks leave bn_stats moments (per bank, 2 chunks) to recover later
    acc = pp2.tile([128, NCC], F32)
    stats = pp2.tile([128, 3, NBGRP, 2, 6], F32)

    def issue_wb(upto):
        while wb_offs and wb_offs[0] < upto:
            woff = wb_offs.pop(0)
            ww = min(WBSEG, DCP - woff)
            nc.sync.dma_start(
                out=wb_s[:, woff : woff + ww],
                in_=wb[:, woff : woff + ww],
            )

    # ---- phase 2: main loop over target segments / psum banks ----
    issue_wb(2 * WBSEG)  # wb segs 0-1 up front
    for s0 in range(0, NCC, SEG):
        g = min(SEG, NCC - s0)
        sb0, sg = s0 // 2, min(SEG, NCC - s0) // 2  # banks in this segment
        t_s = tpool.tile([128, SEG // 2, 2 * ROWS], FP8)
        nc.sync.dma_start(
            out=t_s[:, 0:sg, :],
            in_=tgt[sb0 : sb0 + sg, :, :].rearrange("b p w -> p b w"),
        )
        # keep the wb stream ~2 segments ahead of the mains consumers
        issue_wb((s0 + 2 * SEG) * 128)
        # banks in groups of 3: consecutive injects share one identity
        # LDWEIGHTS (bass elides repeated same-weights loads)
        for gb0 in range(0, g, 6):
            group = []
            for b0 in range(gb0, min(gb0 + 6, g), 2):
                dps = dpsum.tile([128, 512], F32)
                nc.tensor.matmul(
                    dps,
                    lhsT=ident,
                    rhs=t_s[:, b0 // 2, :],
                    start=True,
                    stop=False,
                )
                group.append((b0, dps))
            for b0, dps in group:
                for c in range(2):
                    ch = s0 + b0 + c
                    nc.tensor.matmul(
                        dps[:, c * ROWS : (c + 1) * ROWS],
                        lhsT=wb_s[:, ch * 128 : (ch + 1) * 128],
                        rhs=lhsT_r,
                        start=False,
                        stop=(c == 1),
                    )
            for b0, dps in group:
                bank = (s0 + b0) // 2
                rb = bank % 5
                if rb in ACT_BRES:
                    for c in range(2):
                        ch = s0 + b0 + c
                        pch = dps[:, c * ROWS : (c + 1) * ROWS]
                        nc.scalar.activation(
                            pch, pch, AF.Square, accum_out=acc[:, ch : ch + 1]
                        )
                else:
                    ri = BN_BRES.index(rb)
                    for c in range(2):
                        nc.vector.bn_stats(
                            stats[:, ri, bank // 5, c, :],
                            dps[:, c * ROWS : (c + 1) * ROWS],
                        )

    # ---- phase 3: epilogue ----
    # bn moment recovery on Pool, scattered into acc at the bn chunk columns;
    # then one weighted reduce covers ACT and bn chunks alike.
    # combo columns: 0 = sse, 4 = sum(logR), 5 = kl_raw (1-3 spare)
    combo = pp2.tile([128, 6], F32)
    nc.gpsimd.memset(combo[:, 1:4], 0.0)
    accv = acc.rearrange("p (g rb k) -> p g rb k", rb=5, k=2)
    tb1 = pp2.tile([128, NBGRP, 2], F32)
    tb2 = pp2.tile([128, NBGRP, 2], F32)
    for ri, rb in enumerate(BN_BRES):
        me = stats[:, ri, :, :, 1]
        m2e = stats[:, ri, :, :, 2]
        mo = stats[:, ri, :, :, 4]
        m2o = stats[:, ri, :, :, 5]
        nc.gpsimd.tensor_mul(tb1, me, me)
        nc.gpsimd.tensor_mul(tb2, mo, mo)
        nc.gpsimd.tensor_add(tb1, tb1, tb2)
        nc.gpsimd.tensor_add(tb2, m2e, m2o)
        # sq = 128*(me^2+mo^2) + (m2e+m2o)
        nc.vector.scalar_tensor_tensor(
            accv[:, :, rb, :], tb1, 128.0, tb2, op0=OP.mult, op1=OP.add
        )
    prod = pp2.tile([128, NCC], F32)
    nc.gpsimd.tensor_mul(prod, acc, w150)
    nc.vector.reduce_sum(out=combo[:, 0:1], in_=prod, axis=mybir.AxisListType.X)
    nc.vector.reduce_sum(out=combo[:, 4:5], in_=lrt_s, axis=mybir.AxisListType.X)
    nc.gpsimd.tensor_add(combo[:, 5:6], kl2[:, 0:1], kl2[:, 1:2])

    fps = smallps.tile([6, 1], F32, tag="sps")
    nc.tensor.matmul(fps, lhsT=combo, rhs=ones, start=True, stop=True)
    res = pp2.tile([6, 1], F32)
    nc.scalar.copy(res, fps)
    nc.sync.dma_start(out=out[:].rearrange("(p f) -> p f", f=1), in_=res)


_CACHED_NC = {}


def _get_nc(reps: int = 1):
    if reps not in _CACHED_NC:
        _CACHED_NC[reps] = build_nc(reps)
    return _CACHED_NC[reps]


def make_in_maps(mu_filtered, sigma_filtered, mu_pred, sigma_pred, target,
                 W_dec, b_dec, log_R, eps):
    tgt = np.asarray(target, dtype=np.float32).reshape(ROWS, D_OBS)
    wbf = np.concatenate(
        [np.asarray(W_dec, dtype=np.float32),
         np.asarray(b_dec, dtype=np.float32)[None, :]], axis=0
    )
    lr = np.asarray(log_R, dtype=np.float32)
    smalls = {
        "mu_f": np.ascontiguousarray(
            np.asarray(mu_filtered, dtype=np.float32).reshape(ROWS, LAT)),
        "sig_f": np.ascontiguousarray(
            np.asarray(sigma_filtered, dtype=np.float32).reshape(ROWS, 4 * Z)),
        "mu_p": np.ascontiguousarray(
            np.asarray(mu_pred, dtype=np.float32).reshape(ROWS, LAT)),
        "sig_p": np.ascontiguousarray(
            np.asarray(sigma_pred, dtype=np.float32).reshape(ROWS, 4 * Z)),
        "eps": np.ascontiguousarray(
            np.asarray(eps, dtype=np.float32).reshape(ROWS, LAT)),
    }
    import ml_dtypes

    bf16 = ml_dtypes.bfloat16
    fp8 = mybir.dt.np(FP8)
    in_maps = []
    for c in range(NCORES):
        sl = slice(c * DC, (c + 1) * DC)
        tgt_t = np.zeros((DCP, ROWS), dtype=fp8)
        tgt_t[:DC] = tgt[:, sl].T.astype(fp8)
        tgt_t = tgt_t.reshape(NBANK, 2, 128, ROWS).transpose(0, 2, 1, 3)
        wbp = np.zeros((LATP, DCP), dtype=fp8)
        wbp[:, :DC] = wbf[:, sl].astype(fp8)
        lrp = np.zeros(DCP, dtype=np.float32)
        lrp[:DC] = lr[sl]
        in_maps.append({
            **smalls,
            "tgt": np.ascontiguousarray(tgt_t.reshape(NBANK, 128, 2 * ROWS)),
            "wb": np.ascontiguousarray(wbp),
            "log_r_t": np.ascontiguousarray(lrp.reshape(NCC, 128).T),
        })
    return in_maps


def combine(results):
    sse = 0.0
    slr = 0.0
    for c in range(NCORES):
        v = results[c]["out"]
        sse += float(v[0]) + float(v[1]) + float(v[2]) + float(v[3])
        slr += float(v[4])
    klraw = float(results[0]["out"][5])
    n_tot = ROWS * D_OBS
    loss_integral = 0.5 * (
        n_tot * math.log(2.0 * math.pi) + 2.0 * ROWS * slr + sse
    ) / B
    loss_kl = 0.5 * (klraw - 2.0 * B * T * Z) / B
    return np.float32(loss_integral + loss_kl)


def kernel(mu_filtered, sigma_filtered, mu_pred, sigma_pred, target,
           W_dec, b_dec, log_R, eps):
    nc = _get_nc(1)
    in_maps = make_in_maps(mu_filtered, sigma_filtered, mu_pred, sigma_pred,
                           target, W_dec, b_dec, log_R, eps)
    res = run_bass_kernel_spmd(nc, in_maps, core_ids=list(range(NCORES)))
    return combine(res.results)


# revision 43
# speedup vs baseline: 1.3630x; 1.0145x over previous
"""Trainium2 Bass kernel for the DeepBayesianFilterBlockDiag loss.

Strategy (8-core SPMD, observation-axis sharded, TRANSPOSED layout):
  - The 152064-dim observation axis is split into 8 shards of 19008 columns,
    padded to 19200 = 150*128 per core.  The HOST pre-transposes each core's
    target shard to [75, 128, 512] (bank, d-in-chunk, chunk-pair x row)
    fp8-e4m3 so the observation axis lands on SBUF/PSUM partitions with
    512B-contiguous DMA lines; log_R arrives as [128, 150]; W_dec||b_dec as
    [65, 19200] fp8.  1-byte operands cut HBM traffic (the hard floor) 4x
    vs f32; the resulting ~5e-4 loss error is far inside the 2e-2
    tolerance.
  - Per core:
      * phase 1: Xe = [mu_f + chol(sigma_f) @ eps, 1] and the KL terms
        (tiny per-(b,t,z) 2x2 algebra, split over Pool/DVE so the two
        row-group chains run in parallel); -Xe^T [65,256] fp8 is the
        moving operand of the decode GEMM.
      * main loop over 75 PSUM banks (2 d-chunks each): PE injects the
        fp8 target chunk-pair into the bank with one identity matmul,
        then accumulates -Xe @ W' per 128-d chunk (W' fp8 slices as
        stationary, -Xe^T fp8 moving).  Residual d^T = t - rec sits in PSUM with d on
        partitions.  Banks alternate (period 5): 2/5 of banks use ACT
        Square+accum_out per chunk (in-place in PSUM, per-partition sums
        land directly in acc columns); 3/5 use DVE bn_stats per chunk,
        whose moments are recovered to sums-of-squares in the epilogue
        (sum d^2 = M2_even + 128*mean_even^2 + M2_odd + 128*mean_odd^2)
        and scattered into the same acc columns.
      * epilogue: sse = sum(acc * exp(-2 log_R)) via one weighted reduce,
        plus sum(log_R) and the KL partial; a PE ones-matmul reduces the
        [128,6] combo over partitions; out is a [6] vector.
  - Host combines the 8 partial vectors into the final scalar loss.
"""

import math

import numpy as np

import concourse.bass as bass
import concourse.mybir as mybir
import concourse.tile as tile
from concourse.bass_utils import run_bass_kernel_spmd
from concourse.masks import make_identity

F32 = mybir.dt.float32
F32R = mybir.dt.float32r
BF16 = mybir.dt.bfloat16
FP8 = mybir.dt.float8e4
AF = mybir.ActivationFunctionType
OP = mybir.AluOpType

B, T, Z, DIM = 4, 64, 32, 2
ROWS = B * T          # 256
LAT = Z * DIM         # 64
LATP = LAT + 1        # 65 (ones row folds in b_dec)
D_OBS = 152064
NCORES = 8
DC = D_OBS // NCORES  # 19008 obs columns per core
NCC = 150             # 128-wide d-chunks per core (19200 = padded)
DCP = NCC * 128       # 19200
SEG = 16              # d-chunks per target DMA segment (8 banks)
NBANK = NCC // 2      # 75 psum banks of 2 chunks
# bank residues (mod 5): {0,1} -> ACT square+accum per chunk,
# {2,3,4} -> one DVE bn_stats per bank
ACT_BRES = (0, 1)
BN_BRES = (2, 3, 4)
NBGRP = NBANK // 5    # 15 bank groups of 5
TP_BUFS = 3
DPS_BUFS = 6

MAX_DRAIN_WAITS = 1


def _split_multi_waits(nc, max_waits=1):
    """walrus' per-instruction sync encoding only fits one wait; move extra
    waits emitted by Tile onto NOPs inserted just before the instruction on
    the same engine (same semantics: engine blocks on all of them in order).
    """
    k = 0
    for f in nc.m.functions:
        for blk in f.blocks:
            il = blk.instructions
            i = 0
            while i < len(il):
                inst = il[i]
                si = inst.sync_info
                if si is not None and len(si.on_wait) > max_waits:
                    waits = list(si.on_wait)
                    inst.sync_info = mybir.SyncInfo(
                        on_wait=waits[-max_waits:], on_update=list(si.on_update)
                    )
                    extra = waits[:-max_waits]
                    for j in range(0, len(extra), max_waits):
                        nop = mybir.InstEventSemaphore(
                            name=f"{inst.name}-w{k}",
                            engine=inst.engine,
                            sync_info=mybir.SyncInfo(
                                on_wait=extra[j : j + max_waits], on_update=[]
                            ),
                        )
                        k += 1
                        il.insert(i, nop)
                        i += 1
                i += 1


def _comp4(t, mg, idx):
    # [128, 2, 128] tile -> [128, 32] view of 2x2-block component idx
    return t[:, mg, :].rearrange("p (z k) -> p z k", k=4)[:, :, idx]


def _comp2(t, mg, idx):
    return t[:, mg, :].rearrange("p (z k) -> p z k", k=2)[:, :, idx]


def build_nc(reps: int = 1, split_waits: bool = True, dup: int = 1):
    nc = bass.Bass("TRN2")
    tgt = nc.dram_tensor("tgt", [NBANK, 128, 2 * ROWS], FP8, kind="ExternalInput")
    wb = nc.dram_tensor("wb", [LATP, DCP], FP8, kind="ExternalInput")
    lrt = nc.dram_tensor("log_r_t", [128, NCC], F32, kind="ExternalInput")
    muf = nc.dram_tensor("mu_f", [ROWS, LAT], F32, kind="ExternalInput")
    sgf = nc.dram_tensor("sig_f", [ROWS, 4 * Z], F32, kind="ExternalInput")
    mup = nc.dram_tensor("mu_p", [ROWS, LAT], F32, kind="ExternalInput")
    sgp = nc.dram_tensor("sig_p", [ROWS, 4 * Z], F32, kind="ExternalInput")
    eps = nc.dram_tensor("eps", [ROWS, LAT], F32, kind="ExternalInput")
    out = nc.dram_tensor("out", [6], F32, kind="ExternalOutput")

    with tile.TileContext(nc) as tc:
        with (
            tc.tile_pool(name="big", bufs=1) as big,
            tc.tile_pool(name="tp", bufs=TP_BUFS) as tpool,
            tc.tile_pool(name="small", bufs=1) as small,
            tc.tile_pool(name="pp2", bufs=2) as pp2,
            tc.tile_pool(name="dps", bufs=DPS_BUFS, space="PSUM") as dpsum,
            tc.tile_pool(name="smallps", bufs=1, space="PSUM") as smallps,
        ):
            # loop-invariant constants, built once
            identf = small.tile([128, 128], F32)
            make_identity(nc, identf)
            ident = small.tile([128, 128], FP8)
            nc.gpsimd.tensor_copy(ident, identf)
            ones = small.tile([128, 1], F32)
            nc.vector.memset(ones, 1.0)
            consts = (identf, ident, ones)
            if reps == 1:
                for _ in range(dup):
                    _body(nc, tc, big, tpool, small, pp2, dpsum, smallps, consts,
                          tgt, wb, lrt, muf, sgf, mup, sgp, eps, out)
            else:
                with tc.For_i(0, reps, 1):
                    for _ in range(dup):
                        _body(nc, tc, big, tpool, small, pp2, dpsum, smallps, consts,
                              tgt, wb, lrt, muf, sgf, mup, sgp, eps, out)
    if split_waits:
        # needed for the walrus/HW path; CoreSim wants the raw form
        _split_multi_waits(nc)
    return nc


def _body(nc, tc, big, tpool, small, pp2, dpsum, smallps, consts,
          tgt, wb, lrt, muf, sgf, mup, sgp, eps, out):
    identf, ident, ones = consts

    # ---- small inputs (SWDGE on the idle Pool queue: issued early so the
    # next rep's phase 1 can overlap this rep's main loop) ----
    sigf_s = small.tile([128, 2, 4 * Z], F32)
    sigp_s = small.tile([128, 2, 4 * Z], F32)
    muf_s = small.tile([128, 2, LAT], F32)
    mup_s = small.tile([128, 2, LAT], F32)
    eps_s = small.tile([128, 2, LAT], F32)
    for mg in range(2):
        ve = nc.gpsimd if mg == 0 else nc.vector
        rs = slice(mg * 128, (mg + 1) * 128)
        nc.sync.dma_start(out=sigf_s[:, mg, :], in_=sgf[rs, :])
        nc.sync.dma_start(out=sigp_s[:, mg, :], in_=sgp[rs, :])
        nc.sync.dma_start(out=muf_s[:, mg, :], in_=muf[rs, :])
        nc.sync.dma_start(out=mup_s[:, mg, :], in_=mup[rs, :])
        nc.sync.dma_start(out=eps_s[:, mg, :], in_=eps[rs, :])
    lrt_s = pp2.tile([128, NCC], F32)
    nc.sync.dma_start(out=lrt_s, in_=lrt[:, :])

    # ---- phase 1: Xe (cholesky sample) + KL, per 128-row group.
    # W' arrives sign-flipped from the host, so the GEMM moving operand is
    # +Xe^T: the PE transpose result copies straight to fp8, no negate ----
    lhsT_r = pp2.tile([LATP, 256], FP8)
    nc.gpsimd.memset(lhsT_r[LAT:LATP, :], 1.0)
    kl2 = pp2.tile([128, 2], F32)

    for mg in range(2):
        af = _comp4(sigf_s, mg, 0)
        bf = _comp4(sigf_s, mg, 1)
        cf = _comp4(sigf_s, mg, 2)
        df = _comp4(sigf_s, mg, 3)
        aq = _comp4(sigp_s, mg, 0)
        bq = _comp4(sigp_s, mg, 1)
        cq = _comp4(sigp_s, mg, 2)
        dq = _comp4(sigp_s, mg, 3)

        # cholesky: l11 = sqrt(a); l21 = c/l11; l22 = sqrt(d - l21^2)
        l11 = small.tile([128, Z], F32)
        nc.scalar.sqrt(l11, af)
        r11 = small.tile([128, Z], F32)
        nc.vector.reciprocal(r11, l11)
        l21 = small.tile([128, Z], F32)
        ve.tensor_mul(l21, cf, r11)
        tmp0 = small.tile([128, Z], F32)
        ve.tensor_mul(tmp0, l21, l21)
        ve.tensor_sub(tmp0, df, tmp0)
        l22 = small.tile([128, Z], F32)
        nc.scalar.sqrt(l22, tmp0)

        e1 = _comp2(eps_s, mg, 0)
        e2 = _comp2(eps_s, mg, 1)
        m1 = _comp2(muf_s, mg, 0)
        m2 = _comp2(muf_s, mg, 1)

        xew = small.tile([128, LAT], F32)
        x1v = xew.rearrange("p (z k) -> p z k", k=2)[:, :, 0]
        x2v = xew.rearrange("p (z k) -> p z k", k=2)[:, :, 1]
        tA = small.tile([128, Z], F32)
        ve.tensor_mul(tA, l11, e1)
        ve.tensor_add(x1v, tA, m1)
        tB = small.tile([128, Z], F32)
        ve.tensor_mul(tB, l21, e1)
        tC = small.tile([128, Z], F32)
        ve.tensor_mul(tC, l22, e2)
        ve.tensor_add(tB, tB, tC)
        ve.tensor_add(x2v, tB, m2)

        tps = smallps.tile([LAT, 128], F32, tag="sps")
        nc.tensor.transpose(tps, xew, identf)
        nc.scalar.copy(lhsT_r[0:LAT, mg * 128 : (mg + 1) * 128], tps)

        # KL pieces
        detq = small.tile([128, Z], F32)
        tD = small.tile([128, Z], F32)
        ve.tensor_mul(detq, aq, dq)
        ve.tensor_mul(tD, bq, cq)
        ve.tensor_sub(detq, detq, tD)
        detp = small.tile([128, Z], F32)
        ve.tensor_mul(detp, af, df)
        ve.tensor_mul(tD, bf, cf)
        ve.tensor_sub(detp, detp, tD)
        rdq = small.tile([128, Z], F32)
        nc.vector.reciprocal(rdq, detq)

        # trace numerator: dq*af - bq*bf - cq*cf + aq*df
        tn = small.tile([128, Z], F32)
        ve.tensor_mul(tn, dq, af)
        ve.tensor_mul(tD, aq, df)
        ve.tensor_add(tn, tn, tD)
        ve.tensor_mul(tD, bq, bf)
        ve.tensor_sub(tn, tn, tD)
        ve.tensor_mul(tD, cq, cf)
        ve.tensor_sub(tn, tn, tD)

        # quad numerator: dq*d1^2 - (bq+cq)*d1*d2 + aq*d2^2
        p1 = _comp2(mup_s, mg, 0)
        p2 = _comp2(mup_s, mg, 1)
        d1 = small.tile([128, Z], F32)
        ve.tensor_sub(d1, p1, m1)
        d2 = small.tile([128, Z], F32)
        ve.tensor_sub(d2, p2, m2)
        qn = small.tile([128, Z], F32)
        ve.tensor_mul(tD, d1, d1)
        ve.tensor_mul(qn, dq, tD)
        ve.tensor_mul(tD, d2, d2)
        ve.tensor_mul(tD, aq, tD)
        ve.tensor_add(qn, qn, tD)
        ve.tensor_mul(tD, d1, d2)
        tE = small.tile([128, Z], F32)
        ve.tensor_add(tE, bq, cq)
        ve.tensor_mul(tD, tD, tE)
        ve.tensor_sub(qn, qn, tD)

        klv = small.tile([128, Z], F32)
        ve.tensor_add(klv, tn, qn)
        ve.tensor_mul(klv, klv, rdq)
        # + ln(detq) - ln(detp)
        nc.scalar.activation(tD, detq, AF.Ln)
        ve.tensor_add(klv, klv, tD)
        nc.scalar.activation(tD, detp, AF.Ln)
        ve.tensor_sub(klv, klv, tD)
        nc.vector.reduce_sum(out=kl2[:, mg : mg + 1], in_=klv, axis=mybir.AxisListType.X)

    # w = exp(-2 log_R), already [128, NCC] on partitions
    w150 = pp2.tile([128, NCC], F32)
    nc.scalar.activation(w150, lrt_s, AF.Exp, scale=-2.0)

    # ---- W' (with b_dec row) resident in SBUF as f32r, on the scalar ring,
    # interleaved with the target segments on the same (sync) ring ----
    wb_s = big.tile([LATP, DCP], FP8)
    WBSEG = 3840
    wb_offs = list(range(0, DCP, WBSEG))

    # per-chunk sums of squares: ACT banks accumulate into acc columns; DVE
    # banks leave bn_stats moments (per bank, 2 chunks) to recover later
    acc = pp2.tile([128, NCC], F32)
    stats = pp2.tile([128, 3, NBGRP, 2, 6], F32)

    def issue_wb(upto):
        while wb_offs and wb_offs[0] < upto:
            woff = wb_offs.pop(0)
            ww = min(WBSEG, DCP - woff)
            nc.sync.dma_start(
                out=wb_s[:, woff : woff + ww],
                in_=wb[:, woff : woff + ww],
            )

    # ---- phase 2: main loop over target segments / psum banks ----
    issue_wb(2 * WBSEG)  # wb segs 0-1 up front
    for s0 in range(0, NCC, SEG):
        g = min(SEG, NCC - s0)
        sb0, sg = s0 // 2, min(SEG, NCC - s0) // 2  # banks in this segment
        t_s = tpool.tile([128, SEG // 2, 2 * ROWS], FP8)
        nc.sync.dma_start(
            out=t_s[:, 0:sg, :],
            in_=tgt[sb0 : sb0 + sg, :, :].rearrange("b p w -> p b w"),
        )
        # keep the wb stream ~2 segments ahead of the mains consumers
        issue_wb((s0 + 2 * SEG) * 128)
        # banks in groups of 3: consecutive injects share one identity
        # LDWEIGHTS (bass elides repeated same-weights loads)
        for gb0 in range(0, g, 6):
            group = []
            for b0 in range(gb0, min(gb0 + 6, g), 2):
                dps = dpsum.tile([128, 512], F32)
                nc.tensor.matmul(
                    dps,
                    lhsT=ident,
                    rhs=t_s[:, b0 // 2, :],
                    start=True,
                    stop=False,
                )
                group.append((b0, dps))
            for b0, dps in group:
                for c in range(2):
                    ch = s0 + b0 + c
                    nc.tensor.matmul(
                        dps[:, c * ROWS : (c + 1) * ROWS],
                        lhsT=wb_s[:, ch * 128 : (ch + 1) * 128],
                        rhs=lhsT_r,
                        start=False,
                        stop=(c == 1),
                    )
            for b0, dps in group:
                bank = (s0 + b0) // 2
                rb = bank % 5
                if rb in ACT_BRES:
                    for c in range(2):
                        ch = s0 + b0 + c
                        pch = dps[:, c * ROWS : (c + 1) * ROWS]
                        nc.scalar.activation(
                            pch, pch, AF.Square, accum_out=acc[:, ch : ch + 1]
                        )
                else:
                    ri = BN_BRES.index(rb)
                    for c in range(2):
                        nc.vector.bn_stats(
                            stats[:, ri, bank // 5, c, :],
                            dps[:, c * ROWS : (c + 1) * ROWS],
                        )

    # ---- phase 3: epilogue ----
    # bn moment recovery on Pool, scattered into acc at the bn chunk columns;
    # then one weighted reduce covers ACT and bn chunks alike.
    # combo columns: 0 = sse, 4 = sum(logR), 5 = kl_raw (1-3 spare)
    combo = pp2.tile([128, 6], F32)
    nc.gpsimd.memset(combo[:, 1:4], 0.0)
    accv = acc.rearrange("p (g rb k) -> p g rb k", rb=5, k=2)
    tb1 = pp2.tile([128, NBGRP, 2], F32)
    tb2 = pp2.tile([128, NBGRP, 2], F32)
    for ri, rb in enumerate(BN_BRES):
        me = stats[:, ri, :, :, 1]
        m2e = stats[:, ri, :, :, 2]
        mo = stats[:, ri, :, :, 4]
        m2o = stats[:, ri, :, :, 5]
        nc.gpsimd.tensor_mul(tb1, me, me)
        nc.gpsimd.tensor_mul(tb2, mo, mo)
        nc.gpsimd.tensor_add(tb1, tb1, tb2)
        nc.gpsimd.tensor_add(tb2, m2e, m2o)
        # sq = 128*(me^2+mo^2) + (m2e+m2o)
        nc.vector.scalar_tensor_tensor(
            accv[:, :, rb, :], tb1, 128.0, tb2, op0=OP.mult, op1=OP.add
        )
    prod = pp2.tile([128, NCC], F32)
    nc.gpsimd.tensor_mul(prod, acc, w150)
    nc.vector.reduce_sum(out=combo[:, 0:1], in_=prod, axis=mybir.AxisListType.X)
    nc.vector.reduce_sum(out=combo[:, 4:5], in_=lrt_s, axis=mybir.AxisListType.X)
    nc.gpsimd.tensor_add(combo[:, 5:6], kl2[:, 0:1], kl2[:, 1:2])

    fps = smallps.tile([6, 1], F32, tag="sps")
    nc.tensor.matmul(fps, lhsT=combo, rhs=ones, start=True, stop=True)
    res = pp2.tile([6, 1], F32)
    nc.scalar.copy(res, fps)
    nc.sync.dma_start(out=out[:].rearrange("(p f) -> p f", f=1), in_=res)


_CACHED_NC = {}


def _get_nc(reps: int = 1):
    if reps not in _CACHED_NC:
        _CACHED_NC[reps] = build_nc(reps)
    return _CACHED_NC[reps]


def make_in_maps(mu_filtered, sigma_filtered, mu_pred, sigma_pred, target,
                 W_dec, b_dec, log_R, eps):
    tgt = np.asarray(target, dtype=np.float32).reshape(ROWS, D_OBS)
    wbf = np.concatenate(
        [np.asarray(W_dec, dtype=np.float32),
         np.asarray(b_dec, dtype=np.float32)[None, :]], axis=0
    )
    lr = np.asarray(log_R, dtype=np.float32)
    smalls = {
        "mu_f": np.ascontiguousarray(
            np.asarray(mu_filtered, dtype=np.float32).reshape(ROWS, LAT)),
        "sig_f": np.ascontiguousarray(
            np.asarray(sigma_filtered, dtype=np.float32).reshape(ROWS, 4 * Z)),
        "mu_p": np.ascontiguousarray(
            np.asarray(mu_pred, dtype=np.float32).reshape(ROWS, LAT)),
        "sig_p": np.ascontiguousarray(
            np.asarray(sigma_pred, dtype=np.float32).reshape(ROWS, 4 * Z)),
        "eps": np.ascontiguousarray(
            np.asarray(eps, dtype=np.float32).reshape(ROWS, LAT)),
    }
    import ml_dtypes

    bf16 = ml_dtypes.bfloat16
    fp8 = mybir.dt.np(FP8)
    in_maps = []
    for c in range(NCORES):
        sl = slice(c * DC, (c + 1) * DC)
        tgt_t = np.zeros((DCP, ROWS), dtype=fp8)
        tgt_t[:DC] = tgt[:, sl].T.astype(fp8)
        tgt_t = tgt_t.reshape(NBANK, 2, 128, ROWS).transpose(0, 2, 1, 3)
        wbp = np.zeros((LATP, DCP), dtype=fp8)
        wbp[:, :DC] = (-wbf[:, sl]).astype(fp8)
        lrp = np.zeros(DCP, dtype=np.float32)
        lrp[:DC] = lr[sl]
        in_maps.append({
            **smalls,
            "tgt": np.ascontiguousarray(tgt_t.reshape(NBANK, 128, 2 * ROWS)),
            "wb": np.ascontiguousarray(wbp),
            "log_r_t": np.ascontiguousarray(lrp.reshape(NCC, 128).T),
        })
    return in_maps


def combine(results):
    sse = 0.0
    slr = 0.0
    for c in range(NCORES):
        v = results[c]["out"]
        sse += float(v[0]) + float(v[1]) + float(v[2]) + float(v[3])
        slr += float(v[4])
    klraw = float(results[0]["out"][5])
    n_tot = ROWS * D_OBS
    loss_integral = 0.5 * (
        n_tot * math.log(2.0 * math.pi) + 2.0 * ROWS * slr + sse
    ) / B
    loss_kl = 0.5 * (klraw - 2.0 * B * T * Z) / B
    return np.float32(loss_integral + loss_kl)


def kernel(mu_filtered, sigma_filtered, mu_pred, sigma_pred, target,
           W_dec, b_dec, log_R, eps):
    nc = _get_nc(1)
    in_maps = make_in_maps(mu_filtered, sigma_filtered, mu_pred, sigma_pred,
                           target, W_dec, b_dec, log_R, eps)
    res = run_bass_kernel_spmd(nc, in_maps, core_ids=list(range(NCORES)))
    return combine(res.results)
